# revision 37
# baseline (speedup 1.0000x reference)
"""Trainium2 Bass kernel for nn_CosmosPatcher3d.

Computes the Cosmos 3D Haar wavelet patcher: input [1,3,33,704,704] fp32,
temporal causal pad (first frame repeated 4x -> 36 frames), then two full
3D Haar DWT levels. Equivalent to a separable +-1 Hadamard transform over
4x4x4 blocks scaled by 1/64, producing [1,192,9,176,176] fp32 with channel
layout ch = 96*T2 + 48*H2 + 24*W2 + 12*T1 + 6*H1 + 3*W1 + c.

v4 strategy (8 NeuronCores, shard along H: 704 = 8*88):
- Host converts x to single bf16 (err ~2e-3 << 2e-2 budget) packed as
  [T, H, C, W] so one 3-dim DMA per (t, chunk) loads all channels with
  4224B descriptors, HBM outer dim = h (24..32 blocks -> 16 engines).
- TensorE: fused T+H transform AND W-level-1 butterfly: sums via lhsT=S
  on even/odd stride-2 rhs slices accumulated in PSUM; diffs via S then
  -S. M = th2*32 + y'*4 + th1. Each c gets a full PSUM bank.
- W-level-2: ScalarE/VectorE copy the even-parity PSUM slice to SBUF
  (TensorTensor allows only one PSUM operand), VectorE does add/sub.
- Out tile free layout (w1, c, w2, x) so the store's SBUF-side strides
  (th1@2112, w1@1056, c@352) nest uniformly -> 3-dim AP with HBM outer
  dim i(24): HWDGE spreads stores over all 16 engines. SWDGE (gpsimd)
  takes a share for even balance.
"""

import ml_dtypes
import numpy as np

import concourse.bacc as bacc
import concourse.mybir as mybir
import concourse.tile as tile
from concourse.bass_utils import run_bass_kernel_spmd

N_CORES = 8
C = 3            # input channels
T_IN = 33        # input frames
H_IN = 704       # input height (global)
W_IN = 704       # input width
H_SH = H_IN // N_CORES      # 88 input rows per core
T_OUT = 9
Y_SH = H_SH // 4            # 22 output rows per core
X_OUT = W_IN // 4           # 176
XH = W_IN // 2              # 352 = level-1 output width
CHUNKS = [(0, 32), (32, 32), (64, 24)]

_F32 = mybir.dt.float32
_BF16 = mybir.dt.bfloat16
_BF16_NP = ml_dtypes.bfloat16


def _sgn1d(pos, b2, b1):
    """Composite 2-level Haar sign for position pos in 0..3 (+-1)."""
    s1 = 1.0 if b1 == 0 else (1.0 - 2.0 * (pos % 2))
    s2 = 1.0 if b2 == 0 else (1.0 - 2.0 * (pos // 2))
    return s1 * s2


def _build_signs():
    """bf16 sign matrices including the global 1/64 scale (exact in bf16).

    Rows k = hh*4 + dt (h-major so the load's HBM AP outer dim is h).
    Cols m = (T2*2+H2)*32 + y'*4 + (T1*2+H1)   (y' = hh//4).
    s32 [128,128] / s24 [96,128]: t>=1.  t32 [32,128] / t24 [24,128]:
    t=0 (frame 0 repeated 4x -> only T2=T1=0 subbands, weight 4).
    """
    def mk(nh, t0):
        k = nh if t0 else 4 * nh
        ny = nh // 4
        s = np.zeros((k, 16 * ny), dtype=np.float32)
        for hh in range(nh):
            yp, hp = hh // 4, hh % 4
            for t2 in range(2):
                for h2 in range(2):
                    for t1 in range(2):
                        for h1 in range(2):
                            # M packed: th2 blocks are 4*ny wide (no gaps
                            # for the 24-row chunk -> 96 valid rows)
                            col = (t2 * 2 + h2) * 4 * ny + yp * 4 + (t1 * 2 + h1)
                            sh = _sgn1d(hp, h2, h1)
                            if t0:
                                if t2 == 0 and t1 == 0:
                                    s[hh, col] = 4.0 * sh / 64.0
                            else:
                                for dt in range(4):
                                    st = _sgn1d(dt, t2, t1)
                                    s[hh * 4 + dt, col] = st * sh / 64.0
        return s.astype(_BF16_NP)

    return mk(32, False), mk(24, False), mk(32, True), mk(24, True)


def _build_nc():
    nc = bacc.Bacc(
        "TRN2", target_bir_lowering=False, debug=False, num_devices=N_CORES
    )
    # host packs x as [T, H, C, p1, p2, W/4] with w = 4*x'' + 2*p2 + p1:
    # level-1 matmul rhs slices (p1) are contiguous AND the psum columns
    # come out as (p2, x'') so level-2 reads are unit-stride too
    x = nc.dram_tensor(
        "x", [T_IN, H_SH, C, 2, 2, X_OUT], _BF16, kind="ExternalInput"
    ).ap()
    sg = {}
    for nm, shp in [
        ("s32", [128, 128]), ("s24", [96, 96]),
        ("t32", [32, 128]), ("t24", [24, 96]),
        ("s32n", [128, 128]), ("s24n", [96, 96]),
        ("t32n", [32, 128]), ("t24n", [24, 96]),
    ]:
        sg[nm] = nc.dram_tensor(nm, shp, _BF16, kind="ExternalInput").ap()
    # Packed output: one [128, 2112] f32 tile per (t, chunk), stored as a
    # single contiguous 1.08MB DMA (outer dim 128 -> all 16 engines, 8448B
    # descriptors). The host unpacks to the [192, 9, 176, 176] layout.
    out = nc.dram_tensor(
        "out", [T_OUT * len(CHUNKS), 128, 2112], _F32, kind="ExternalOutput"
    ).ap()

    with tile.TileContext(nc) as tc:
        with (
            tc.tile_pool(name="signs", bufs=1) as sgp,
            tc.tile_pool(name="rhs", bufs=8) as rhp,
            tc.tile_pool(name="even", bufs=6) as evp,
            tc.tile_pool(name="outp", bufs=10) as otp,
            tc.tile_pool(name="psum", bufs=2, space="PSUM") as psp,
        ):
            st = {}
            for nm in ("s32", "s24", "t32", "t24", "s32n", "s24n", "t32n", "t24n"):
                t_ = sgp.tile(list(sg[nm].shape), _BF16, tag=nm)
                nc.sync.dma_start(out=t_, in_=sg[nm])
                st[nm] = t_

            store_i = 0
            for t in range(T_OUT):
                for ci, (h0, nh) in enumerate(CHUNKS):
                    ny = nh // 4
                    kdim = nh if t == 0 else 4 * nh
                    mdim = 4 * nh   # M packed: 128 or 96 valid rows
                    if t == 0:
                        lp = st["t32"] if nh == 32 else st["t24"]
                        ln = st["t32n"] if nh == 32 else st["t24n"]
                    else:
                        lp = st["s32"] if nh == 32 else st["s24"]
                        ln = st["s32n"] if nh == 32 else st["s24n"]

                    # one load per (t, chunk): partitions k = hh*4 + dt
                    rv = rhp.tile([128, C, 2, XH], _BF16, tag="rhs")
                    if t == 0:
                        src = x[0, h0 : h0 + nh].rearrange(
                            "h c p q w -> h (c p q w)"
                        )
                    else:
                        src = x[4 * t - 3 : 4 * t + 1, h0 : h0 + nh].rearrange(
                            "t h c p q w -> h t (c p q w)"
                        )
                    nc.sync.dma_start(
                        out=rv[:kdim].rearrange("k c p w -> k (c p w)"),
                        in_=src,
                    )

                    # W-level-1 sums in PE: ps_s[m, c, x'] = S @ (even+odd).
                    # Each c gets a full 512-f32 PSUM bank so accumulation
                    # groups never straddle bank boundaries.
                    ps_s = psp.tile([128, C, 512], _F32, tag="ps")
                    for c in range(C):
                        nc.tensor.matmul(
                            ps_s[:mdim, c, :XH], lp, rv[:kdim, c, 0],
                            start=True, stop=False,
                        )
                        nc.tensor.matmul(
                            ps_s[:mdim, c, :XH], lp, rv[:kdim, c, 1],
                            start=False, stop=True,
                        )

                    # W-level-2 from sums -> out tile (w2, w1, c, x).
                    # psum cols are (p2, x''): both slices unit-stride.
                    ot = otp.tile([128, 2, 2, C, X_OUT], _F32, tag="ot")
                    s_o = ps_s[:mdim, :, X_OUT : 2 * X_OUT]
                    se = evp.tile([128, C, X_OUT], _F32, tag="ev")
                    nc.scalar.copy(out=se[:mdim], in_=ps_s[:mdim, :, :X_OUT])
                    nc.vector.tensor_add(
                        out=ot[:mdim, 0, 0], in0=se[:mdim], in1=s_o
                    )
                    nc.vector.tensor_sub(
                        out=ot[:mdim, 1, 0], in0=se[:mdim], in1=s_o
                    )

                    # W-level-1 diffs in PE: ps_d = S @ even + (-S) @ odd
                    ps_d = psp.tile([128, C, 512], _F32, tag="ps")
                    for c in range(C):
                        nc.tensor.matmul(
                            ps_d[:mdim, c, :XH], lp, rv[:kdim, c, 0],
                            start=True, stop=False,
                        )
                    for c in range(C):
                        nc.tensor.matmul(
                            ps_d[:mdim, c, :XH], ln, rv[:kdim, c, 1],
                            start=False, stop=True,
                        )

                    d_o = ps_d[:mdim, :, X_OUT : 2 * X_OUT]
                    de = evp.tile([128, C, X_OUT], _F32, tag="ev")
                    nc.scalar.copy(out=de[:mdim], in_=ps_d[:mdim, :, :X_OUT])
                    nc.vector.tensor_add(
                        out=ot[:mdim, 0, 1], in0=de[:mdim], in1=d_o
                    )
                    nc.vector.tensor_sub(
                        out=ot[:mdim, 1, 1], in0=de[:mdim], in1=d_o
                    )

                    # packed contiguous store per (t, chunk), split in two
                    # halves issued on scalar (HWDGE) and gpsimd (SWDGE) in
                    # parallel; loads keep sync's FIFO ring to themselves
                    half = mdim // 2
                    ofl = ot[:mdim].rearrange("m a b c x -> m (a b c x)")
                    nc.scalar.dma_start(
                        out=out[t * len(CHUNKS) + ci, :half], in_=ofl[:half]
                    )
                    nc.gpsimd.dma_start(
                        out=out[t * len(CHUNKS) + ci, half:mdim],
                        in_=ofl[half:],
                    )

    nc.compile()
    return nc


_NC_CACHE = None


def _prep_inputs(hs):
    """Shard along H, convert to single bf16, pack as [T, H, C, W]."""
    s32, s24, t32, t24 = _build_signs()
    base = {
        "s32": s32, "s24": s24, "t32": t32, "t24": t24,
        "s32n": -s32, "s24n": -s24, "t32n": -t32, "t24n": -t24,
    }
    in_maps = []
    for k in range(N_CORES):
        xk = hs[0, :, :, k * H_SH : (k + 1) * H_SH, :]  # [C, T, H, W]
        xk = xk.transpose(1, 2, 0, 3)                    # [T, H, C, W]
        # w = 4*x'' + 2*p2 + p1 -> [T, H, C, p1, p2, x'']
        xk = xk.reshape(T_IN, H_SH, C, X_OUT, 2, 2).transpose(0, 1, 2, 5, 4, 3)
        xk = np.ascontiguousarray(xk).astype(_BF16_NP)
        m = dict(base)
        m["x"] = xk
        in_maps.append(m)
    return in_maps


def kernel(hidden_states: np.ndarray) -> np.ndarray:
    global _NC_CACHE
    if _NC_CACHE is None:
        _NC_CACHE = _build_nc()
    nc = _NC_CACHE

    hs = np.asarray(hidden_states, dtype=np.float32)
    assert hs.shape == (1, C, T_IN, H_IN, W_IN), hs.shape
    in_maps = _prep_inputs(hs)

    res = run_bass_kernel_spmd(nc, in_maps, core_ids=list(range(N_CORES)))

    out = np.empty((1, 192, T_OUT, H_IN // 4, X_OUT), dtype=np.float32)
    # unpack [27, 128, 2112] -> [192, 9, 22, 176] per core:
    # m = th2*(4*ny) + yp*4 + th1, f = w2*1056 + w1*528 + c*176 + x,
    # ch = 48*th2 + 24*w2 + 6*th1 + 3*w1 + c
    ov = out[0].reshape(4, 2, 4, 2, C, T_OUT, H_IN // 4, X_OUT)
    for k in range(N_CORES):
        arr = np.asarray(res.results[k]["out"]).reshape(
            T_OUT, len(CHUNKS), 128, 2, 2, C, X_OUT
        )  # [t, ci, m, w2, w1, c, x]
        for ci, (h0, nh) in enumerate(CHUNKS):
            ny, y0 = nh // 4, h0 // 4
            a = arr[:, ci, : 16 * ny].reshape(
                T_OUT, 4, ny, 4, 2, 2, C, X_OUT
            )  # [t, th2, yp, th1, w2, w1, c, x]
            ov[:, :, :, :, :, :, k * Y_SH + y0 : k * Y_SH + y0 + ny, :] = (
                a.transpose(1, 4, 3, 5, 6, 0, 2, 7)
            )
    return out


# revision 38
# speedup vs baseline: 1.1394x; 1.1394x over previous
"""Trainium2 Bass kernel for nn_CosmosPatcher3d.

Computes the Cosmos 3D Haar wavelet patcher: input [1,3,33,704,704] fp32,
temporal causal pad (first frame repeated 4x -> 36 frames), then two full
3D Haar DWT levels. Equivalent to a separable +-1 Hadamard transform over
4x4x4 blocks scaled by 1/64, producing [1,192,9,176,176] fp32 with channel
layout ch = 96*T2 + 48*H2 + 24*W2 + 12*T1 + 6*H1 + 3*W1 + c.

v4 strategy (8 NeuronCores, shard along H: 704 = 8*88):
- Host converts x to single bf16 (err ~2e-3 << 2e-2 budget) packed as
  [T, H, C, W] so one 3-dim DMA per (t, chunk) loads all channels with
  4224B descriptors, HBM outer dim = h (24..32 blocks -> 16 engines).
- TensorE: fused T+H transform AND W-level-1 butterfly: sums via lhsT=S
  on even/odd stride-2 rhs slices accumulated in PSUM; diffs via S then
  -S. M = th2*32 + y'*4 + th1. Each c gets a full PSUM bank.
- W-level-2: ScalarE/VectorE copy the even-parity PSUM slice to SBUF
  (TensorTensor allows only one PSUM operand), VectorE does add/sub.
- Out tile free layout (w1, c, w2, x) so the store's SBUF-side strides
  (th1@2112, w1@1056, c@352) nest uniformly -> 3-dim AP with HBM outer
  dim i(24): HWDGE spreads stores over all 16 engines. SWDGE (gpsimd)
  takes a share for even balance.
"""

import ml_dtypes
import numpy as np

import concourse.bacc as bacc
import concourse.mybir as mybir
import concourse.tile as tile
from concourse.bass_utils import run_bass_kernel_spmd

N_CORES = 8
C = 3            # input channels
T_IN = 33        # input frames
H_IN = 704       # input height (global)
W_IN = 704       # input width
H_SH = H_IN // N_CORES      # 88 input rows per core
T_OUT = 9
Y_SH = H_SH // 4            # 22 output rows per core
X_OUT = W_IN // 4           # 176
XH = W_IN // 2              # 352 = level-1 output width
CHUNKS = [(0, 32), (32, 32), (64, 24)]

_F32 = mybir.dt.float32
_BF16 = mybir.dt.bfloat16
_BF16_NP = ml_dtypes.bfloat16


def _sgn1d(pos, b2, b1):
    """Composite 2-level Haar sign for position pos in 0..3 (+-1)."""
    s1 = 1.0 if b1 == 0 else (1.0 - 2.0 * (pos % 2))
    s2 = 1.0 if b2 == 0 else (1.0 - 2.0 * (pos // 2))
    return s1 * s2


def _build_signs():
    """bf16 sign matrices including the global 1/64 scale (exact in bf16).

    Rows k = hh*4 + dt (h-major so the load's HBM AP outer dim is h).
    Cols m = (T2*2+H2)*32 + y'*4 + (T1*2+H1)   (y' = hh//4).
    s32 [128,128] / s24 [96,128]: t>=1.  t32 [32,128] / t24 [24,128]:
    t=0 (frame 0 repeated 4x -> only T2=T1=0 subbands, weight 4).
    """
    def mk(nh, t0):
        k = nh if t0 else 4 * nh
        ny = nh // 4
        s = np.zeros((k, 16 * ny), dtype=np.float32)
        for hh in range(nh):
            yp, hp = hh // 4, hh % 4
            for t2 in range(2):
                for h2 in range(2):
                    for t1 in range(2):
                        for h1 in range(2):
                            # M packed: th2 blocks are 4*ny wide (no gaps
                            # for the 24-row chunk -> 96 valid rows)
                            col = (t2 * 2 + h2) * 4 * ny + yp * 4 + (t1 * 2 + h1)
                            sh = _sgn1d(hp, h2, h1)
                            if t0:
                                if t2 == 0 and t1 == 0:
                                    s[hh, col] = 4.0 * sh / 64.0
                            else:
                                for dt in range(4):
                                    st = _sgn1d(dt, t2, t1)
                                    s[hh * 4 + dt, col] = st * sh / 64.0
        return s.astype(_BF16_NP)

    return mk(32, False), mk(24, False), mk(32, True), mk(24, True)


def _build_nc():
    nc = bacc.Bacc(
        "TRN2", target_bir_lowering=False, debug=False, num_devices=N_CORES
    )
    # host packs x as [T, H, C, p1, p2, W/4] with w = 4*x'' + 2*p2 + p1:
    # level-1 matmul rhs slices (p1) are contiguous AND the psum columns
    # come out as (p2, x'') so level-2 reads are unit-stride too
    x = nc.dram_tensor(
        "x", [T_IN, H_SH, C, 2, 2, X_OUT], _BF16, kind="ExternalInput"
    ).ap()
    sg = {}
    for nm, shp in [
        ("s32", [128, 128]), ("s24", [96, 96]),
        ("t32", [32, 128]), ("t24", [24, 96]),
        ("s32n", [128, 128]), ("s24n", [96, 96]),
        ("t32n", [32, 128]), ("t24n", [24, 96]),
    ]:
        sg[nm] = nc.dram_tensor(nm, shp, _BF16, kind="ExternalInput").ap()
    # Packed output: one [128, 2112] f32 tile per (t, chunk), stored as a
    # single contiguous 1.08MB DMA (outer dim 128 -> all 16 engines, 8448B
    # descriptors). The host unpacks to the [192, 9, 176, 176] layout.
    out = nc.dram_tensor(
        "out", [T_OUT * len(CHUNKS), 128, 2112], _F32, kind="ExternalOutput"
    ).ap()

    with tile.TileContext(nc) as tc:
        with (
            tc.tile_pool(name="signs", bufs=1) as sgp,
            tc.tile_pool(name="rhs", bufs=8) as rhp,
            tc.tile_pool(name="even", bufs=6) as evp,
            tc.tile_pool(name="outp", bufs=10) as otp,
            tc.tile_pool(name="psum", bufs=2, space="PSUM") as psp,
        ):
            st = {}
            for nm in ("s32", "s24", "t32", "t24", "s32n", "s24n", "t32n", "t24n"):
                t_ = sgp.tile(list(sg[nm].shape), _BF16, tag=nm)
                nc.sync.dma_start(out=t_, in_=sg[nm])
                st[nm] = t_

            store_i = 0
            for t in range(T_OUT):
                for ci, (h0, nh) in enumerate(CHUNKS):
                    ny = nh // 4
                    kdim = nh if t == 0 else 4 * nh
                    mdim = 4 * nh   # M packed: 128 or 96 valid rows
                    if t == 0:
                        lp = st["t32"] if nh == 32 else st["t24"]
                        ln = st["t32n"] if nh == 32 else st["t24n"]
                    else:
                        lp = st["s32"] if nh == 32 else st["s24"]
                        ln = st["s32n"] if nh == 32 else st["s24n"]

                    # one load per (t, chunk): partitions k = hh*4 + dt
                    rv = rhp.tile([128, C, 2, XH], _BF16, tag="rhs")
                    if t == 0:
                        src = x[0, h0 : h0 + nh].rearrange(
                            "h c p q w -> h (c p q w)"
                        )
                    else:
                        src = x[4 * t - 3 : 4 * t + 1, h0 : h0 + nh].rearrange(
                            "t h c p q w -> h t (c p q w)"
                        )
                    nc.sync.dma_start(
                        out=rv[:kdim].rearrange("k c p w -> k (c p w)"),
                        in_=src,
                    )

                    # W-level-1 sums in PE: ps_s[m, c, x'] = S @ (even+odd).
                    # Each c gets a full 512-f32 PSUM bank so accumulation
                    # groups never straddle bank boundaries.
                    ps_s = psp.tile([128, C, 512], _F32, tag="ps")
                    for c in range(C):
                        nc.tensor.matmul(
                            ps_s[:mdim, c, :XH], lp, rv[:kdim, c, 0],
                            start=True, stop=False,
                        )
                        nc.tensor.matmul(
                            ps_s[:mdim, c, :XH], lp, rv[:kdim, c, 1],
                            start=False, stop=True,
                        )

                    # W-level-2 from sums -> out tile (w2, w1, c, x).
                    # psum cols are (p2, x''): both slices unit-stride.
                    ot = otp.tile([128, 2, 2, C, X_OUT], _F32, tag="ot")
                    s_o = ps_s[:mdim, :, X_OUT : 2 * X_OUT]
                    se = evp.tile([128, C, X_OUT], _F32, tag="ev")
                    nc.scalar.copy(out=se[:mdim], in_=ps_s[:mdim, :, :X_OUT])
                    nc.vector.tensor_add(
                        out=ot[:mdim, 0, 0], in0=se[:mdim], in1=s_o
                    )
                    nc.vector.tensor_sub(
                        out=ot[:mdim, 1, 0], in0=se[:mdim], in1=s_o
                    )

                    # W-level-1 diffs in PE: ps_d = S @ even + (-S) @ odd
                    ps_d = psp.tile([128, C, 512], _F32, tag="ps")
                    for c in range(C):
                        nc.tensor.matmul(
                            ps_d[:mdim, c, :XH], lp, rv[:kdim, c, 0],
                            start=True, stop=False,
                        )
                    for c in range(C):
                        nc.tensor.matmul(
                            ps_d[:mdim, c, :XH], ln, rv[:kdim, c, 1],
                            start=False, stop=True,
                        )

                    d_o = ps_d[:mdim, :, X_OUT : 2 * X_OUT]
                    de = evp.tile([128, C, X_OUT], _F32, tag="ev")
                    nc.scalar.copy(out=de[:mdim], in_=ps_d[:mdim, :, :X_OUT])
                    nc.vector.tensor_add(
                        out=ot[:mdim, 0, 1], in0=de[:mdim], in1=d_o
                    )
                    nc.vector.tensor_sub(
                        out=ot[:mdim, 1, 1], in0=de[:mdim], in1=d_o
                    )

                    # one packed contiguous store per (t, chunk); alternate
                    # scalar (HWDGE) / gpsimd (SWDGE) so stores don't
                    # serialize behind the loads on sync's FIFO ring
                    eng = [nc.scalar, nc.gpsimd][store_i % 2]
                    store_i += 1
                    eng.dma_start(
                        out=out[t * len(CHUNKS) + ci, :mdim],
                        in_=ot[:mdim].rearrange("m a b c x -> m (a b c x)"),
                    )

    nc.compile()
    return nc


_NC_CACHE = None


def _prep_inputs(hs):
    """Shard along H, convert to single bf16, pack as [T, H, C, W]."""
    s32, s24, t32, t24 = _build_signs()
    base = {
        "s32": s32, "s24": s24, "t32": t32, "t24": t24,
        "s32n": -s32, "s24n": -s24, "t32n": -t32, "t24n": -t24,
    }
    in_maps = []
    for k in range(N_CORES):
        xk = hs[0, :, :, k * H_SH : (k + 1) * H_SH, :]  # [C, T, H, W]
        xk = xk.transpose(1, 2, 0, 3)                    # [T, H, C, W]
        # w = 4*x'' + 2*p2 + p1 -> [T, H, C, p1, p2, x'']
        xk = xk.reshape(T_IN, H_SH, C, X_OUT, 2, 2).transpose(0, 1, 2, 5, 4, 3)
        xk = np.ascontiguousarray(xk).astype(_BF16_NP)
        m = dict(base)
        m["x"] = xk
        in_maps.append(m)
    return in_maps


def kernel(hidden_states: np.ndarray) -> np.ndarray:
    global _NC_CACHE
    if _NC_CACHE is None:
        _NC_CACHE = _build_nc()
    nc = _NC_CACHE

    hs = np.asarray(hidden_states, dtype=np.float32)
    assert hs.shape == (1, C, T_IN, H_IN, W_IN), hs.shape
    in_maps = _prep_inputs(hs)

    res = run_bass_kernel_spmd(nc, in_maps, core_ids=list(range(N_CORES)))

    out = np.empty((1, 192, T_OUT, H_IN // 4, X_OUT), dtype=np.float32)
    # unpack [27, 128, 2112] -> [192, 9, 22, 176] per core:
    # m = th2*(4*ny) + yp*4 + th1, f = w2*1056 + w1*528 + c*176 + x,
    # ch = 48*th2 + 24*w2 + 6*th1 + 3*w1 + c
    ov = out[0].reshape(4, 2, 4, 2, C, T_OUT, H_IN // 4, X_OUT)
    for k in range(N_CORES):
        arr = np.asarray(res.results[k]["out"]).reshape(
            T_OUT, len(CHUNKS), 128, 2, 2, C, X_OUT
        )  # [t, ci, m, w2, w1, c, x]
        for ci, (h0, nh) in enumerate(CHUNKS):
            ny, y0 = nh // 4, h0 // 4
            a = arr[:, ci, : 16 * ny].reshape(
                T_OUT, 4, ny, 4, 2, 2, C, X_OUT
            )  # [t, th2, yp, th1, w2, w1, c, x]
            ov[:, :, :, :, :, :, k * Y_SH + y0 : k * Y_SH + y0 + ny, :] = (
                a.transpose(1, 4, 3, 5, 6, 0, 2, 7)
            )
    return out


# revision 41
# speedup vs baseline: 1.1867x; 1.0415x over previous
"""Trainium2 Bass kernel for nn_CosmosPatcher3d.

Computes the Cosmos 3D Haar wavelet patcher: input [1,3,33,704,704] fp32,
temporal causal pad (first frame repeated 4x -> 36 frames), then two full
3D Haar DWT levels. Equivalent to a separable +-1 Hadamard transform over
4x4x4 blocks scaled by 1/64, producing [1,192,9,176,176] fp32 with channel
layout ch = 96*T2 + 48*H2 + 24*W2 + 12*T1 + 6*H1 + 3*W1 + c.

Strategy (8 NeuronCores, shard along H: 704 = 8*88; 426us -> ~159us):
- Host converts x to single bf16 (err ~2e-3 << 2e-2 budget) packed as
  [T, H, C, p1, p2, W/4] with w = 4x''+2p2+p1: both W-butterfly levels
  are pre-deinterleaved so every on-chip access is unit-stride. One
  3-dim DMA per (t, chunk) loads all channels (4224B descriptors, HBM
  outer dim = h so HWDGE spreads over all 16 SDMA engines).
- TensorE: fused T+H transform AND W-level-1 butterfly: sums via lhsT=S
  accumulating even+odd rhs halves in PSUM; diffs via S then -S.
  M = th2*(4ny) + y'*4 + th1 (packed; 96 rows for the 24-row chunk).
  Each c gets a full 512-f32 PSUM bank (accumulation groups must not
  straddle bank boundaries).
- W-level-2: ScalarE copies the p2=0 PSUM half to SBUF (TensorTensor
  allows only one PSUM operand; GpSimd cannot touch PSUM), VectorE does
  the 4 add/subs.
- Stores: one packed contiguous [mdim, 8448B] DMA per (t, chunk) into a
  scratch DRAM tensor, alternating scalar (HWDGE) / gpsimd (SWDGE)
  queues; loads keep sync's FIFO ring. All 16 engines stay evenly
  loaded (~2.5MB each); the host unpacks to the reference layout.
"""

import ml_dtypes
import numpy as np

import concourse.bacc as bacc
import concourse.mybir as mybir
import concourse.tile as tile
from concourse.bass_utils import run_bass_kernel_spmd

N_CORES = 8
C = 3            # input channels
T_IN = 33        # input frames
H_IN = 704       # input height (global)
W_IN = 704       # input width
H_SH = H_IN // N_CORES      # 88 input rows per core
T_OUT = 9
Y_SH = H_SH // 4            # 22 output rows per core
X_OUT = W_IN // 4           # 176
XH = W_IN // 2              # 352 = level-1 output width
CHUNKS = [(0, 32), (32, 32), (64, 24)]

_F32 = mybir.dt.float32
_BF16 = mybir.dt.bfloat16
_BF16_NP = ml_dtypes.bfloat16


def _sgn1d(pos, b2, b1):
    """Composite 2-level Haar sign for position pos in 0..3 (+-1)."""
    s1 = 1.0 if b1 == 0 else (1.0 - 2.0 * (pos % 2))
    s2 = 1.0 if b2 == 0 else (1.0 - 2.0 * (pos // 2))
    return s1 * s2


def _build_signs():
    """bf16 sign matrices including the global 1/64 scale (exact in bf16).

    Rows k = hh*4 + dt (h-major so the load's HBM AP outer dim is h).
    Cols m = (T2*2+H2)*32 + y'*4 + (T1*2+H1)   (y' = hh//4).
    s32 [128,128] / s24 [96,128]: t>=1.  t32 [32,128] / t24 [24,128]:
    t=0 (frame 0 repeated 4x -> only T2=T1=0 subbands, weight 4).
    """
    def mk(nh, t0):
        k = nh if t0 else 4 * nh
        ny = nh // 4
        s = np.zeros((k, 16 * ny), dtype=np.float32)
        for hh in range(nh):
            yp, hp = hh // 4, hh % 4
            for t2 in range(2):
                for h2 in range(2):
                    for t1 in range(2):
                        for h1 in range(2):
                            # M packed: th2 blocks are 4*ny wide (no gaps
                            # for the 24-row chunk -> 96 valid rows)
                            col = (t2 * 2 + h2) * 4 * ny + yp * 4 + (t1 * 2 + h1)
                            sh = _sgn1d(hp, h2, h1)
                            if t0:
                                if t2 == 0 and t1 == 0:
                                    s[hh, col] = 4.0 * sh / 64.0
                            else:
                                for dt in range(4):
                                    st = _sgn1d(dt, t2, t1)
                                    s[hh * 4 + dt, col] = st * sh / 64.0
        return s.astype(_BF16_NP)

    return mk(32, False), mk(24, False), mk(32, True), mk(24, True)


def _build_nc():
    nc = bacc.Bacc(
        "TRN2", target_bir_lowering=False, debug=False, num_devices=N_CORES
    )
    # host packs x as [T, H, C, p1, p2, W/4] with w = 4*x'' + 2*p2 + p1:
    # level-1 matmul rhs slices (p1) are contiguous AND the psum columns
    # come out as (p2, x'') so level-2 reads are unit-stride too
    x = nc.dram_tensor(
        "x", [T_IN, H_SH, C, 2, 2, X_OUT], _BF16, kind="ExternalInput"
    ).ap()
    sg = {}
    for nm, shp in [
        ("s32", [128, 128]), ("s24", [96, 96]),
        ("t32", [32, 128]), ("t24", [24, 96]),
        ("s32n", [128, 128]), ("s24n", [96, 96]),
        ("t32n", [32, 128]), ("t24n", [24, 96]),
    ]:
        sg[nm] = nc.dram_tensor(nm, shp, _BF16, kind="ExternalInput").ap()
    # Packed output: one [128, 2112] f32 tile per (t, chunk), stored as a
    # single contiguous 1.08MB DMA (outer dim 128 -> all 16 engines, 8448B
    # descriptors). The host unpacks to the [192, 9, 176, 176] layout.
    out = nc.dram_tensor(
        "out", [T_OUT * len(CHUNKS), 128, 2112], _F32, kind="ExternalOutput"
    ).ap()

    with tile.TileContext(nc) as tc:
        with (
            tc.tile_pool(name="signs", bufs=1) as sgp,
            tc.tile_pool(name="rhs", bufs=8) as rhp,
            tc.tile_pool(name="even", bufs=6) as evp,
            tc.tile_pool(name="outp", bufs=10) as otp,
            tc.tile_pool(name="psum", bufs=2, space="PSUM") as psp,
        ):
            st = {}
            for nm in ("s32", "s24", "t32", "t24", "s32n", "s24n", "t32n", "t24n"):
                t_ = sgp.tile(list(sg[nm].shape), _BF16, tag=nm)
                nc.sync.dma_start(out=t_, in_=sg[nm])
                st[nm] = t_

            store_i = 0
            for t in range(T_OUT):
                for ci, (h0, nh) in enumerate(CHUNKS):
                    ny = nh // 4
                    kdim = nh if t == 0 else 4 * nh
                    mdim = 4 * nh   # M packed: 128 or 96 valid rows
                    if t == 0:
                        lp = st["t32"] if nh == 32 else st["t24"]
                        ln = st["t32n"] if nh == 32 else st["t24n"]
                    else:
                        lp = st["s32"] if nh == 32 else st["s24"]
                        ln = st["s32n"] if nh == 32 else st["s24n"]

                    # one load per (t, chunk): partitions k = hh*4 + dt
                    rv = rhp.tile([128, C, 2, XH], _BF16, tag="rhs")
                    if t == 0:
                        src = x[0, h0 : h0 + nh].rearrange(
                            "h c p q w -> h (c p q w)"
                        )
                    else:
                        src = x[4 * t - 3 : 4 * t + 1, h0 : h0 + nh].rearrange(
                            "t h c p q w -> h t (c p q w)"
                        )
                    nc.sync.dma_start(
                        out=rv[:kdim].rearrange("k c p w -> k (c p w)"),
                        in_=src,
                    )

                    # Pass 1: e = S @ even into PSUM. Snapshot what the
                    # d-bands need (a = e0+e1, c = e0-e1) while the odd
                    # half accumulates in place: s = e + S @ odd. Then
                    # ot01 = 2a - ot00 and ot11 = 2c - ot10 reproduce the
                    # diff-bands with half the matmul work.
                    ps = psp.tile([128, C, 512], _F32, tag="ps")
                    for c in range(C):
                        nc.tensor.matmul(
                            ps[:mdim, c, :XH], lp, rv[:kdim, c, 0],
                            start=True, stop=False,
                        )
                    e1 = ps[:mdim, :, X_OUT : 2 * X_OUT]
                    ee = evp.tile([128, C, X_OUT], _F32, tag="ev")
                    eo = evp.tile([128, C, X_OUT], _F32, tag="ev")
                    av = evp.tile([128, C, X_OUT], _F32, tag="ev")
                    cv = evp.tile([128, C, X_OUT], _F32, tag="ev")
                    nc.scalar.copy(out=ee[:mdim], in_=ps[:mdim, :, :X_OUT])
                    nc.scalar.copy(out=eo[:mdim], in_=e1)
                    nc.gpsimd.tensor_add(
                        out=av[:mdim], in0=ee[:mdim], in1=eo[:mdim]
                    )
                    nc.gpsimd.tensor_sub(
                        out=cv[:mdim], in0=ee[:mdim], in1=eo[:mdim]
                    )

                    for c in range(C):
                        nc.tensor.matmul(
                            ps[:mdim, c, :XH], lp, rv[:kdim, c, 1],
                            start=False, stop=True,
                        )

                    # W-level-2 from sums -> out tile (w2, w1, c, x)
                    ot = otp.tile([128, 2, 2, C, X_OUT], _F32, tag="ot")
                    s_o = ps[:mdim, :, X_OUT : 2 * X_OUT]
                    se = evp.tile([128, C, X_OUT], _F32, tag="ev")
                    nc.scalar.copy(out=se[:mdim], in_=ps[:mdim, :, :X_OUT])
                    nc.vector.tensor_add(
                        out=ot[:mdim, 0, 0], in0=se[:mdim], in1=s_o
                    )
                    nc.vector.tensor_sub(
                        out=ot[:mdim, 1, 0], in0=se[:mdim], in1=s_o
                    )
                    nc.vector.scalar_tensor_tensor(
                        out=ot[:mdim, 0, 1], in0=av[:mdim], scalar=2.0,
                        in1=ot[:mdim, 0, 0],
                        op0=mybir.AluOpType.mult,
                        op1=mybir.AluOpType.subtract,
                    )
                    nc.vector.scalar_tensor_tensor(
                        out=ot[:mdim, 1, 1], in0=cv[:mdim], scalar=2.0,
                        in1=ot[:mdim, 1, 0],
                        op0=mybir.AluOpType.mult,
                        op1=mybir.AluOpType.subtract,
                    )

                    # one packed contiguous store per (t, chunk); alternate
                    # scalar (HWDGE) / gpsimd (SWDGE) so stores don't
                    # serialize behind the loads on sync's FIFO ring
                    eng = [nc.scalar, nc.gpsimd][store_i % 2]
                    store_i += 1
                    eng.dma_start(
                        out=out[t * len(CHUNKS) + ci, :mdim],
                        in_=ot[:mdim].rearrange("m a b c x -> m (a b c x)"),
                    )

    nc.compile()
    return nc


_NC_CACHE = None


def _prep_inputs(hs):
    """Shard along H, convert to single bf16, pack as [T, H, C, W]."""
    s32, s24, t32, t24 = _build_signs()
    base = {
        "s32": s32, "s24": s24, "t32": t32, "t24": t24,
        "s32n": -s32, "s24n": -s24, "t32n": -t32, "t24n": -t24,
    }
    in_maps = []
    for k in range(N_CORES):
        xk = hs[0, :, :, k * H_SH : (k + 1) * H_SH, :]  # [C, T, H, W]
        xk = xk.transpose(1, 2, 0, 3)                    # [T, H, C, W]
        # w = 4*x'' + 2*p2 + p1 -> [T, H, C, p1, p2, x'']
        xk = xk.reshape(T_IN, H_SH, C, X_OUT, 2, 2).transpose(0, 1, 2, 5, 4, 3)
        xk = np.ascontiguousarray(xk).astype(_BF16_NP)
        m = dict(base)
        m["x"] = xk
        in_maps.append(m)
    return in_maps


def kernel(hidden_states: np.ndarray) -> np.ndarray:
    global _NC_CACHE
    if _NC_CACHE is None:
        _NC_CACHE = _build_nc()
    nc = _NC_CACHE

    hs = np.asarray(hidden_states, dtype=np.float32)
    assert hs.shape == (1, C, T_IN, H_IN, W_IN), hs.shape
    in_maps = _prep_inputs(hs)

    res = run_bass_kernel_spmd(nc, in_maps, core_ids=list(range(N_CORES)))

    out = np.empty((1, 192, T_OUT, H_IN // 4, X_OUT), dtype=np.float32)
    # unpack [27, 128, 2112] -> [192, 9, 22, 176] per core:
    # m = th2*(4*ny) + yp*4 + th1, f = w2*1056 + w1*528 + c*176 + x,
    # ch = 48*th2 + 24*w2 + 6*th1 + 3*w1 + c
    ov = out[0].reshape(4, 2, 4, 2, C, T_OUT, H_IN // 4, X_OUT)
    for k in range(N_CORES):
        arr = np.asarray(res.results[k]["out"]).reshape(
            T_OUT, len(CHUNKS), 128, 2, 2, C, X_OUT
        )  # [t, ci, m, w2, w1, c, x]
        for ci, (h0, nh) in enumerate(CHUNKS):
            ny, y0 = nh // 4, h0 // 4
            a = arr[:, ci, : 16 * ny].reshape(
                T_OUT, 4, ny, 4, 2, 2, C, X_OUT
            )  # [t, th2, yp, th1, w2, w1, c, x]
            ov[:, :, :, :, :, :, k * Y_SH + y0 : k * Y_SH + y0 + ny, :] = (
                a.transpose(1, 4, 3, 5, 6, 0, 2, 7)
            )
    return out
